# revision 18
# baseline (speedup 1.0000x reference)
# Conv2d 3x3 VALID stride-1 on 8 TRN2 NeuronCores via 1D Winograd F(2,3)
# along W (direct 3-tap accumulation along H).
#
# Problem: x[32,128,56,56] f32, weight[256,128,3,3] f32, bias[256] f32
#          -> out[32,256,54,54] f32
#
# Sharding: data-parallel over batch — 4 images per core, weight replicated.
#
# Algorithm (per core, per image, per oc-half, per 18-output-row chunk):
#   W-axis Winograd F(2,3): output col pair (2tc, 2tc+1) from input cols
#   d_j = x[:, :, 2tc+j], j=0..3:
#     V0 = d0-d2, V1 = d1+d2, V2 = d2-d1, V3 = d1-d3           (DVE, fp16)
#     M_q = sum_kh U[q,kh] @ V_q[rows r0+kh : r0+kh+18]        (PE, 4 PSUM
#           tiles [128,18,27], 3 matmuls each, N=486 cols)
#     U0 = w0, U1 = (w0+w1+w2)/2, U2 = (w0-w1+w2)/2, U3 = w2   (host numpy)
#     out[:, :, 0::2] = M0 + M1 + M2 + bias
#     out[:, :, 1::2] = M1 - M2 - M3 + bias
#   The A-transform runs on ACT+DVE: ACT evicts M1,M2 to fp16 and
#   M3n = -M3+bias; DVE does t0=M1+M2, out0=(M0+bias)+t0 (scalar_tensor_
#   tensor, M0 read from PSUM), t1=M1-M2, out1=t1+M3n, writing stride-2
#   interleaved into a [128,18,54] f32 staging tile; one DMA per chunk.
#
# PE cost: 139,968 matmul columns (2/3 of direct conv's 209,952); ACT and
# DVE hide under the PE critical path. Startup: x0 rows 0..19 + weights DMA
# first; dummy matmuls keep the PE busy from t~0 so the p-state ramp
# (full speed needs 3us of continuous PE busy) completes before real work.
# The final (img,och) block tapers into 9-row chunks with split drains so
# the kernel tail is a short pipeline, not an 18-row chunk + big DMA.

import numpy as np

import concourse.tile as tile
from concourse import bacc, mybir
from concourse.bass_utils import run_bass_kernel_spmd

N_CORES = 8
IMGS = 4          # images per core
IC = 128
OC = 256
H = W = 56
OH = OW = 54
KH = 3
TC = OW // 2      # 27 Winograd column tiles
CR = 18           # output rows per chunk
NCHUNK = OH // CR # 3

FP16 = mybir.dt.float16
F32 = mybir.dt.float32
AF = mybir.ActivationFunctionType
ALU = mybir.AluOpType

N_WARMUP_MM = 60  # dummy matmuls to lift the PE p-state


def build_conv_bass(repeat=1, num_devices=N_CORES, variant=0):
    nc = bacc.Bacc(
        "TRN2",
        target_bir_lowering=False,
        debug=False,
        num_devices=num_devices,
    )
    # x arrives column-deinterleaved (host repack): [..., 0, :] = even cols
    # 0,2..54, [..., 1, :] = odd cols 1,3..55 — so the B-transform reads are
    # contiguous (DVE 2x_1p) instead of stride-2.
    x_ext = nc.dram_tensor("x", [IMGS, IC, H, 2, 28], FP16, kind="ExternalInput")
    ut_ext = nc.dram_tensor("ut", [2, IC, 4, KH, 128], FP16, kind="ExternalInput")
    b_ext = nc.dram_tensor("bias", [128, 2], F32, kind="ExternalInput")
    out_ext = nc.dram_tensor("out", [IMGS, OC, OH, OW], F32, kind="ExternalOutput")

    with tile.TileContext(nc) as tc:
        with (
            tc.tile_pool(name="consts", bufs=1) as cpool,
            tc.tile_pool(name="xin", bufs=1) as xpool,
            tc.tile_pool(name="vbuf", bufs=2) as vpool,
            tc.tile_pool(name="psum", bufs=7, space="PSUM") as ppool,
            tc.tile_pool(name="warm", bufs=1, space="PSUM") as wpsum,
            tc.tile_pool(name="etmp", bufs=10) as epool,
            tc.tile_pool(name="outs", bufs=4) as opool,
        ):
            # PE warm-up: matmuls on a zeroed scratch tile, no DMA deps.
            warm_in = cpool.tile([128, 128], FP16)
            nc.vector.memset(warm_in[:], 0.0)
            warm_ps = wpsum.tile([128, 64], F32)
            for _ in range(N_WARMUP_MM):
                nc.tensor.matmul(warm_ps[:], warm_in[:], warm_in[:, 0:64],
                                 start=True, stop=True)

            # Startup-ordered DMAs: first chunk's deps first.
            x_tiles = [xpool.tile([IC, H, 2, 28], FP16, tag=f"x{i}",
                                  name=f"x{i}")
                       for i in range(IMGS)]
            u_sbs = [cpool.tile([IC, 4, KH, 128], FP16, tag=f"u{och}",
                                name=f"u{och}")
                     for och in range(2)]
            nc.sync.dma_start(x_tiles[0][:, 0:20], x_ext[0, :, 0:20])
            nc.sync.dma_start(u_sbs[0][:], ut_ext[0])
            nc.sync.dma_start(x_tiles[0][:, 20:H], x_ext[0, :, 20:H])
            nc.sync.dma_start(u_sbs[1][:], ut_ext[1])
            b_sb = cpool.tile([128, 2], F32)
            nc.sync.dma_start(b_sb[:], b_ext[:])
            for img in range(1, IMGS):
                nc.sync.dma_start(x_tiles[img][:], x_ext[img])

            for _rep in range(repeat):
              for img in range(IMGS):
                xt = x_tiles[img]
                vt = vpool.tile([IC, 4, H, TC], FP16, tag="v",
                                name=f"v{_rep}_{img}")
                # B-transform on DVE (2x_1p: all-fp16 contiguous reads after
                # the host even/odd repack). gpsimd tensor arithmetic crashes
                # the exec unit on real HW, so everything stays on DVE. Two
                # row pieces so chunk 0 can start early.
                # issue order matches matmul consumption (q=1 first)
                for r0, r1 in ((0, 20), (20, H)):
                    d0 = xt[:, r0:r1, 0, 0:TC]
                    d2 = xt[:, r0:r1, 0, 1:TC + 1]
                    d1 = xt[:, r0:r1, 1, 0:TC]
                    d3 = xt[:, r0:r1, 1, 1:TC + 1]
                    nc.vector.tensor_add(vt[:, 1, r0:r1, :], d1, d2)
                    nc.vector.tensor_sub(vt[:, 2, r0:r1, :], d2, d1)
                    nc.vector.tensor_sub(vt[:, 3, r0:r1, :], d1, d3)
                    nc.vector.tensor_sub(vt[:, 0, r0:r1, :], d0, d2)
                for och in range(2):
                  b_ap = b_sb[:, och:och + 1]
                  ocs = slice(och * 128, (och + 1) * 128)
                  final_blk = (_rep == repeat - 1 and img == IMGS - 1
                               and och == 1)
                  # The final block tapers: its last chunks are 9 rows so
                  # the kernel tail drains a small pipeline, not an 18-row
                  # chunk + full-size DMA.
                  chunks = ([(0, 18), (18, 36), (36, 45), (45, 54)]
                            if final_blk else [(0, 18), (18, 36), (36, 54)])
                  for t, (r0, r1) in enumerate(chunks):
                    cr = r1 - r0
                    is_final = final_blk and t == len(chunks) - 1
                    # q-order (1,2,3,0): M0 is freed latest (by the final
                    # stt), so allocate/compute it last — PSUM buffer reuse
                    # then always lands on an early-freed tile. The final
                    # chunk computes M3 last and drains its consumers
                    # (m3n/out1/DMA) in row halves so the tail pipelines.
                    q_order = (1, 2, 0, 3) if is_final else (1, 2, 3, 0)
                    ms = {}
                    for q in q_order:
                        ms[q] = ppool.tile([128, cr, TC], F32, tag="m",
                                           name=f"m{img}_{och}_{t}_{q}")
                        for kh in range(KH):
                            nc.tensor.matmul(
                                ms[q][:],
                                u_sbs[och][:, q, kh, :],
                                vt[:, q, r0 + kh:r0 + kh + cr, :],
                                start=(kh == 0),
                                stop=(kh == KH - 1),
                            )
                    m1f = epool.tile([128, cr, TC], FP16, tag="e",
                                     name=f"m1f{img}_{och}_{t}")
                    m2f = epool.tile([128, cr, TC], FP16, tag="e",
                                     name=f"m2f{img}_{och}_{t}")
                    m3n = epool.tile([128, cr, TC], FP16, tag="e",
                                     name=f"m3n{img}_{och}_{t}")
                    t0 = epool.tile([128, cr, TC], FP16, tag="e",
                                    name=f"t0{img}_{och}_{t}")
                    t1 = epool.tile([128, cr, TC], FP16, tag="e",
                                    name=f"t1{img}_{och}_{t}")
                    stg = opool.tile([128, cr, OW], F32, tag="ob",
                                     name=f"ob{img}_{och}_{t}")
                    nc.scalar.activation(m1f[:], ms[1][:], AF.Copy)
                    nc.scalar.activation(m2f[:], ms[2][:], AF.Copy)
                    nc.vector.tensor_add(t0[:], m1f[:], m2f[:])
                    nc.vector.tensor_sub(t1[:], m1f[:], m2f[:])
                    if not is_final:
                        nc.scalar.activation(m3n[:], ms[3][:], AF.Identity,
                                             bias=b_ap, scale=-1.0)
                        nc.vector.tensor_add(stg[:, :, 1:54:2], t1[:],
                                             m3n[:])
                        nc.vector.scalar_tensor_tensor(
                            stg[:, :, 0:53:2], ms[0][:], b_ap, t0[:],
                            op0=ALU.add, op1=ALU.add)
                        nc.sync.dma_start(
                            out_ext[img, ocs, r0:r1, :], stg[:])
                    else:
                        # out0 completes while M3's taps still run.
                        nc.vector.scalar_tensor_tensor(
                            stg[:, :, 0:53:2], ms[0][:], b_ap, t0[:],
                            op0=ALU.add, op1=ALU.add)
                        hr = cr // 2
                        for h0, h1 in ((0, hr), (hr, cr)):
                            nc.scalar.activation(
                                m3n[:, h0:h1], ms[3][:, h0:h1], AF.Identity,
                                bias=b_ap, scale=-1.0)
                            nc.vector.tensor_add(
                                stg[:, h0:h1, 1:54:2], t1[:, h0:h1],
                                m3n[:, h0:h1])
                            nc.sync.dma_start(
                                out_ext[img, ocs, r0 + h0:r0 + h1, :],
                                stg[:, h0:h1],
                            )
    nc.compile()
    return nc


_CACHE = {}


def _get_nc(repeat=1, variant=0):
    key = (repeat, variant)
    if key not in _CACHE:
        _CACHE[key] = build_conv_bass(repeat=repeat, variant=variant)
    return _CACHE[key]


def prep_inputs(x, weight, bias):
    """Host-side layout prep. Returns the dict of full-batch device inputs."""
    x = np.asarray(x)
    weight = np.asarray(weight, dtype=np.float32)
    bias = np.asarray(bias, dtype=np.float32)
    assert x.shape == (32, IC, H, W)
    # deinterleave columns: [..., 2, 28] with [..., 0, :]=even, [..., 1, :]=odd
    x16 = np.ascontiguousarray(
        x.astype(np.float16).reshape(32, IC, H, 28, 2).transpose(0, 1, 2, 4, 3))
    w0 = weight[:, :, :, 0]
    w1 = weight[:, :, :, 1]
    w2 = weight[:, :, :, 2]
    U = np.stack([w0, (w0 + w1 + w2) * 0.5, (w0 - w1 + w2) * 0.5, w2])
    # [q, oc, ic, kh] -> [och, ic, q, kh, oc_lo]
    ut = np.ascontiguousarray(
        U.reshape(4, 2, 128, IC, KH).transpose(1, 3, 0, 4, 2)
        .astype(np.float16))
    b2 = np.ascontiguousarray(bias.reshape(2, 128).T)  # [128, 2]
    return {"x": x16, "ut": ut, "bias": b2}


def make_in_maps(x, weight, bias):
    full = prep_inputs(x, weight, bias)
    return [
        {"x": full["x"][i * IMGS:(i + 1) * IMGS], "ut": full["ut"],
         "bias": full["bias"]}
        for i in range(N_CORES)
    ]


def kernel(x, weight, bias, _want_results_obj=False, _repeat=1, **run_kwargs):
    in_maps = make_in_maps(x, weight, bias)
    nc = _get_nc(_repeat)
    res = run_bass_kernel_spmd(nc, in_maps, core_ids=list(range(N_CORES)),
                               **run_kwargs)
    out = np.concatenate([res.results[i]["out"] for i in range(N_CORES)],
                         axis=0)
    if _want_results_obj:
        return out, res
    return out


# revision 36
# speedup vs baseline: 1.0077x; 1.0077x over previous
# Conv2d 3x3 VALID stride-1 on 8 TRN2 NeuronCores via 1D Winograd F(2,3)
# along W (direct 3-tap accumulation along H).
#
# Problem: x[32,128,56,56] f32, weight[256,128,3,3] f32, bias[256] f32
#          -> out[32,256,54,54] f32
#
# Sharding: data-parallel over batch — 4 images per core, weight replicated.
#
# Algorithm (per core, per image, per oc-half, per 18-output-row chunk):
#   W-axis Winograd F(2,3): output col pair (2tc, 2tc+1) from input cols
#   d_j = x[:, :, 2tc+j], j=0..3:
#     V0 = d0-d2, V1 = d1+d2, V2 = d2-d1, V3 = d1-d3           (DVE, fp16)
#     M_q = sum_kh U[q,kh] @ V_q[rows r0+kh : r0+kh+18]        (PE, 4 PSUM
#           tiles [128,18,27], 3 matmuls each, N=486 cols)
#     U0 = w0, U1 = (w0+w1+w2)/2, U2 = (w0-w1+w2)/2, U3 = w2   (host numpy)
#     out[:, :, 0::2] = M0 + M1 + M2 + bias
#     out[:, :, 1::2] = M1 - M2 - M3 + bias
#   The A-transform runs on ACT+DVE: ACT evicts M1,M2 to fp16 and
#   M3n = -M3+bias; DVE does t0=M1+M2, out0=(M0+bias)+t0 (scalar_tensor_
#   tensor, M0 read from PSUM), t1=M1-M2, out1=t1+M3n, writing stride-2
#   interleaved into a [128,18,54] f32 staging tile; one DMA per chunk.
#
# PE cost: 139,968 matmul columns (2/3 of direct conv's 209,952); ACT and
# DVE hide under the PE critical path. Startup: x0 rows 0..19 + weights DMA
# first; dummy matmuls keep the PE busy from t~0 so the p-state ramp
# (full speed needs 3us of continuous PE busy) completes before real work.
# The final (img,och) block tapers into 9-row chunks with split drains so
# the kernel tail is a short pipeline, not an 18-row chunk + big DMA.

import numpy as np

import concourse.tile as tile
from concourse import bacc, mybir
from concourse.bass_utils import run_bass_kernel_spmd

N_CORES = 8
IMGS = 4          # images per core
IC = 128
OC = 256
H = W = 56
OH = OW = 54
KH = 3
TC = OW // 2      # 27 Winograd column tiles
CR = 18           # output rows per chunk
NCHUNK = OH // CR # 3

FP16 = mybir.dt.float16
F32 = mybir.dt.float32
AF = mybir.ActivationFunctionType
ALU = mybir.AluOpType

N_WARMUP_MM = 60  # dummy matmuls to lift the PE p-state

DR0 = 46          # first output row of the final block's direct-conv tail
FINAL_CHUNKS = [(0, 18), (18, 35), (35, DR0)]


def build_conv_bass(repeat=1, num_devices=N_CORES, variant=0):
    nc = bacc.Bacc(
        "TRN2",
        target_bir_lowering=False,
        debug=False,
        num_devices=num_devices,
    )
    # x arrives column-deinterleaved (host repack): [..., 0, :] = even cols
    # 0,2..54, [..., 1, :] = odd cols 1,3..55 — so the B-transform reads are
    # contiguous (DVE 2x_1p) instead of stride-2.
    x_ext = nc.dram_tensor("x", [IMGS, IC, H, 2, 28], FP16, kind="ExternalInput")
    ut_ext = nc.dram_tensor("ut", [2, IC, 4, KH, 128], FP16, kind="ExternalInput")
    wd_ext = nc.dram_tensor("wd", [IC, 2, KH, KH, 128], FP16,
                            kind="ExternalInput")
    b_ext = nc.dram_tensor("bias", [128, 2], F32, kind="ExternalInput")
    out_ext = nc.dram_tensor("out", [IMGS, OC, OH, OW], F32, kind="ExternalOutput")

    with tile.TileContext(nc) as tc:
        with (
            tc.tile_pool(name="consts", bufs=1) as cpool,
            tc.tile_pool(name="xin", bufs=1) as xpool,
            tc.tile_pool(name="vbuf", bufs=2) as vpool,
            tc.tile_pool(name="psum", bufs=7, space="PSUM") as ppool,
            tc.tile_pool(name="warm", bufs=1, space="PSUM") as wpsum,
            tc.tile_pool(name="etmp", bufs=10) as epool,
            tc.tile_pool(name="outs", bufs=4) as opool,
        ):
            # PE warm-up: matmuls on a zeroed scratch tile, no DMA deps.
            warm_in = cpool.tile([128, 128], FP16)
            nc.vector.memset(warm_in[:], 0.0)
            warm_ps = wpsum.tile([128, 64], F32)
            for _ in range(N_WARMUP_MM):
                nc.tensor.matmul(warm_ps[:], warm_in[:], warm_in[:, 0:64],
                                 start=True, stop=True)

            # Startup-ordered DMAs: first chunk's deps first.
            x_tiles = [xpool.tile([IC, H, 2, 28], FP16, tag=f"x{i}",
                                  name=f"x{i}")
                       for i in range(IMGS)]
            u_sbs = [cpool.tile([IC, 4, KH, 128], FP16, tag=f"u{och}",
                                name=f"u{och}")
                     for och in range(2)]
            nc.sync.dma_start(x_tiles[0][:, 0:20], x_ext[0, :, 0:20])
            nc.sync.dma_start(u_sbs[0][:], ut_ext[0])
            nc.sync.dma_start(x_tiles[0][:, 20:H], x_ext[0, :, 20:H])
            nc.sync.dma_start(u_sbs[1][:], ut_ext[1])
            b_sb = cpool.tile([128, 2], F32)
            nc.sync.dma_start(b_sb[:], b_ext[:])
            for img in range(1, IMGS):
                nc.sync.dma_start(x_tiles[img][:], x_ext[img])
            # direct weights for the tail chunk — needed only at the very
            # end, so this DMA goes last
            wd_sb = cpool.tile([IC, 2, KH, KH, 128], FP16)
            nc.sync.dma_start(wd_sb[:], wd_ext[:])

            for _rep in range(repeat):
              for img in range(IMGS):
                xt = x_tiles[img]
                vt = vpool.tile([IC, 4, H, TC], FP16, tag="v",
                                name=f"v{_rep}_{img}")
                # B-transform on DVE (2x_1p: all-fp16 contiguous reads after
                # the host even/odd repack). gpsimd tensor arithmetic crashes
                # the exec unit on real HW, so everything stays on DVE. Two
                # row pieces so chunk 0 can start early.
                # issue order matches matmul consumption (q=1 first)
                for r0, r1 in ((0, 20), (20, H)):
                    d0 = xt[:, r0:r1, 0, 0:TC]
                    d2 = xt[:, r0:r1, 0, 1:TC + 1]
                    d1 = xt[:, r0:r1, 1, 0:TC]
                    d3 = xt[:, r0:r1, 1, 1:TC + 1]
                    nc.vector.tensor_add(vt[:, 1, r0:r1, :], d1, d2)
                    nc.vector.tensor_sub(vt[:, 2, r0:r1, :], d2, d1)
                    nc.vector.tensor_sub(vt[:, 3, r0:r1, :], d1, d3)
                    nc.vector.tensor_sub(vt[:, 0, r0:r1, :], d0, d2)
                for och in range(2):
                  b_ap = b_sb[:, och:och + 1]
                  ocs = slice(och * 128, (och + 1) * 128)
                  final_blk = (_rep == repeat - 1 and img == IMGS - 1
                               and och == 1)
                  # The final block's last 2 output rows are computed by a
                  # DIRECT convolution below: its tail drain is one ACT
                  # eviction + a tiny DMA, with no DVE in the chain.
                  chunks = (FINAL_CHUNKS
                            if final_blk else [(0, 18), (18, 36), (36, 54)])
                  for t, (r0, r1) in enumerate(chunks):
                    cr = r1 - r0
                    # q-order (1,2,3,0): M0 is freed latest (by the final
                    # stt), so allocate/compute it last — PSUM buffer reuse
                    # then always lands on an early-freed tile.
                    ms = {}
                    for q in (1, 2, 3, 0):
                        ms[q] = ppool.tile([128, cr, TC], F32, tag="m",
                                           name=f"m{img}_{och}_{t}_{q}")
                        for kh in range(KH):
                            nc.tensor.matmul(
                                ms[q][:],
                                u_sbs[och][:, q, kh, :],
                                vt[:, q, r0 + kh:r0 + kh + cr, :],
                                start=(kh == 0),
                                stop=(kh == KH - 1),
                            )
                    m1f = epool.tile([128, cr, TC], FP16, tag="e",
                                     name=f"m1f{img}_{och}_{t}")
                    m2f = epool.tile([128, cr, TC], FP16, tag="e",
                                     name=f"m2f{img}_{och}_{t}")
                    t0 = epool.tile([128, cr, TC], FP16, tag="e",
                                    name=f"t0{img}_{och}_{t}")
                    t1 = epool.tile([128, cr, TC], FP16, tag="e",
                                    name=f"t1{img}_{och}_{t}")
                    stg = opool.tile([128, cr, OW], F32, tag="ob",
                                     name=f"ob{img}_{och}_{t}")
                    nc.scalar.activation(m1f[:], ms[1][:], AF.Copy)
                    nc.scalar.activation(m2f[:], ms[2][:], AF.Copy)
                    nc.vector.tensor_add(t0[:], m1f[:], m2f[:])
                    nc.vector.tensor_sub(t1[:], m1f[:], m2f[:])
                    m3n = epool.tile([128, cr, TC], FP16, tag="e",
                                     name=f"m3n{img}_{och}_{t}")
                    nc.scalar.activation(m3n[:], ms[3][:], AF.Identity,
                                         bias=b_ap, scale=-1.0)
                    nc.vector.tensor_add(stg[:, :, 1:54:2], t1[:], m3n[:])
                    nc.vector.scalar_tensor_tensor(
                        stg[:, :, 0:53:2], ms[0][:], b_ap, t0[:],
                        op0=ALU.add, op1=ALU.add)
                    nc.sync.dma_start(out_ext[img, ocs, r0:r1, :], stg[:])
                  if final_blk:
                    # Direct-conv tail for output rows DR0..53: its drain
                    # is ACT-only (no DVE). Taps are split by output-column
                    # parity (x is stored as even/odd planes), accumulating
                    # per-parity PSUM tiles; two strided ACT evictions fold
                    # the bias and restore column order.
                    dr = OH - DR0
                    psd = [ppool.tile([128, dr, TC], F32, tag="m",
                                      name=f"psd{par}") for par in (0, 1)]
                    # (plane, idx_lo) of the moving slice per (parity, kw)
                    mov = {0: [(0, 0), (1, 0), (0, 1)],
                           1: [(1, 0), (0, 1), (1, 1)]}
                    for par in (0, 1):
                        n = 0
                        for kh in range(KH):
                            rows = slice(DR0 + kh, OH + kh)
                            for kw in range(KH):
                                pl, i0 = mov[par][kw]
                                nc.tensor.matmul(
                                    psd[par][:],
                                    wd_sb[:, och, kh, kw, :],
                                    xt[:, rows, pl, i0:i0 + TC],
                                    start=(n == 0), stop=(n == KH * KH - 1),
                                )
                                n += 1
                    stgd = opool.tile([128, dr, OW], F32, tag="ob",
                                      name="obd")
                    nc.scalar.activation(stgd[:, :, 0:53:2], psd[0][:],
                                         AF.Identity, bias=b_ap)
                    nc.scalar.activation(stgd[:, :, 1:54:2], psd[1][:],
                                         AF.Identity, bias=b_ap)
                    nc.sync.dma_start(out_ext[img, ocs, DR0:OH, :], stgd[:])
    nc.compile()
    return nc


_CACHE = {}


def _get_nc(repeat=1, variant=0):
    key = (repeat, variant)
    if key not in _CACHE:
        _CACHE[key] = build_conv_bass(repeat=repeat, variant=variant)
    return _CACHE[key]


def prep_inputs(x, weight, bias):
    """Host-side layout prep. Returns the dict of full-batch device inputs."""
    x = np.asarray(x)
    weight = np.asarray(weight, dtype=np.float32)
    bias = np.asarray(bias, dtype=np.float32)
    assert x.shape == (32, IC, H, W)
    # deinterleave columns: [..., 2, 28] with [..., 0, :]=even, [..., 1, :]=odd
    x16 = np.ascontiguousarray(
        x.astype(np.float16).reshape(32, IC, H, 28, 2).transpose(0, 1, 2, 4, 3))
    w0 = weight[:, :, :, 0]
    w1 = weight[:, :, :, 1]
    w2 = weight[:, :, :, 2]
    U = np.stack([w0, (w0 + w1 + w2) * 0.5, (w0 - w1 + w2) * 0.5, w2])
    # [q, oc, ic, kh] -> [och, ic, q, kh, oc_lo]
    ut = np.ascontiguousarray(
        U.reshape(4, 2, 128, IC, KH).transpose(1, 3, 0, 4, 2)
        .astype(np.float16))
    b2 = np.ascontiguousarray(bias.reshape(2, 128).T)  # [128, 2]
    # direct weights for the tail chunk: [ic, och, kh, kw, oc_lo]
    wd = np.ascontiguousarray(
        weight.reshape(2, 128, IC, KH, KH).transpose(2, 0, 3, 4, 1)
        .astype(np.float16))
    return {"x": x16, "ut": ut, "bias": b2, "wd": wd}


def make_in_maps(x, weight, bias):
    full = prep_inputs(x, weight, bias)
    return [
        {"x": full["x"][i * IMGS:(i + 1) * IMGS], "ut": full["ut"],
         "bias": full["bias"], "wd": full["wd"]}
        for i in range(N_CORES)
    ]


def kernel(x, weight, bias, _want_results_obj=False, _repeat=1, **run_kwargs):
    in_maps = make_in_maps(x, weight, bias)
    nc = _get_nc(_repeat)
    res = run_bass_kernel_spmd(nc, in_maps, core_ids=list(range(N_CORES)),
                               **run_kwargs)
    out = np.concatenate([res.results[i]["out"] for i in range(N_CORES)],
                         axis=0)
    if _want_results_obj:
        return out, res
    return out
